# revision 26
# baseline (speedup 1.0000x reference)
"""Trainium2 Bass kernel for nn_Encoder (NRI-style GNN message-passing encoder).

Reference math:
  h  = MLP1(x)                       [B,N,H]   N=64 nodes, H=128
  e  = MLP2(node2edge(h))            [B,E,H]   E=4032 edges (fully connected)
  n  = MLP3(edge2node(e))            [B,N,H]
  e2 = MLP4([node2edge(n), e])       [B,E,H]
  out= e2 @ wout + bout              [B,E,16]

Distribution: data-parallel over batch, 8 items per core x 8 cores.

Kernel structure (all-bf16, fp32 PSUM accumulation):
- Edge reorder (receiver-major): reordered edge p = (s-1)*64 + j is the
  edge (sender=(j+s)%64, receiver=j), s=1..63.
- node2edge is LOW-RANK: instead of per-edge matmuls, compute per-node
  projections A = W2s.h + b2a, B = W2r.h (64-col matmuls), then expand
  per edge as h2pre[:, (s-1)*64+j] = A2[:, s+j] + B[:, j] with a single
  DVE tensor_tensor add (A2 = [A A]; shifted window via stride-1 outer
  AP, B via stride-0 broadcast AP). ReLU in-place via tensor_scalar_max.
  bf16 operands put DVE in its 2x/4x fast modes.
- edge2node: halving add-tree on h2T (DVE, bf16).
- MLP4 pre-activation accumulated in PSUM: w2bk @ h2T (per-edge matmul)
  + node term. Node term for chunks 0-5 comes from a DVE-built D =
  An2[s+j] + Bn[j] added via identity-stationary matmul; chunks 6-7 use
  I.An2-shifted + I.Bn-bcast matmuls directly (engine load balance).
- x_skip eliminated: w2bk = w2b @ w4a_k and b4a' = b4a + b2b @ w4a_k
  folded host-side (exact algebra). Final linear folded: w4o = w4b@wout,
  bias b4o added on host.
- Output packed 4 chunks per PSUM bank at partition offsets 0/32/64/96
  using stationary [w4o | zeros] [128,32]; DMA per item is one
  [128,1024] bf16 tile.
- 3-stage software pipeline across items (DVE expansion / PE smalls /
  PE big matmuls) so PE, DVE and ACT overlap.

The harness calls kernel(**inputs) with full unsharded inputs.
"""
import sys

sys.path.insert(0, "/opt/trn_rl_repo")

import numpy as np
import ml_dtypes

import concourse.bass as bass
from concourse import bacc
import concourse.mybir as mybir
import concourse.tile as tile
from concourse.bass_utils import run_bass_kernel_spmd

F32 = mybir.dt.float32
BF16 = mybir.dt.bfloat16
BFNP = ml_dtypes.bfloat16

N_NODES = 64
N_EDGES = 4032
BATCH = 64
N_IN = 64
H = 128
N_OUT = 16
N_CORES = 8
B_LOC = BATCH // N_CORES

# chunk c covers s-blocks d0..d0+7 (edge cols (d0-1)*64 .. +512); chunk 7
# overlaps chunk 6 by one block so every chunk is exactly 512 columns.
CHUNKS = [1 + 8 * c for c in range(7)] + [56]
ND = 0          # all chunks expand the node term on PE via I.An2/I.Bn

# wpack layout (bf16): [xT(512) | 12 weight blocks of 128]
WNAMES = ["w1a", "w1b", "w2s", "w2r", "w2b", "w3a", "w3b",
          "w4s", "w4r", "w2bk", "w4oz", "ident"]
XCOLS = B_LOC * N_NODES            # 512
WTOT = XCOLS + len(WNAMES) * H     # 2048
BNAMES = ["b1a", "b1b", "b2a", "b2n", "b3a", "b3b", "b4a"]

_AF = mybir.ActivationFunctionType
_ALU = mybir.AluOpType


def _edge_perm():
    """perm[p] = original edge index of reordered edge p = (s-1)*64 + j,
    which is edge (sender=(j+s)%64, receiver=j)."""
    s, j = np.meshgrid(np.arange(1, 64), np.arange(64), indexing="ij")
    i = (j + s) % 64
    return (i * 63 + (j - (j > i))).reshape(-1)


def _ap(t, off, dims):
    return bass.AP(tensor=t.tensor, offset=t.offset + off, ap=[t.ap[0]] + dims)


def build_kernel():
    nc = bacc.Bacc("TRN2", target_bir_lowering=False, debug=False)

    wpack_d = nc.dram_tensor("wpack", [H, WTOT], BF16, kind="ExternalInput").ap()
    bias_d = nc.dram_tensor("bias", [H, 8], F32, kind="ExternalInput").ap()
    # per item: [128, 1024] bf16; partitions 32k..32k+15 of col-half h hold
    # chunk c = 4h+k (rows 16..31 of each 32 are zeros from the w4oz pad).
    y_d = nc.dram_tensor("y", [B_LOC, H, 1024], BF16, kind="ExternalOutput").ap()

    scale2n = 1.0 / (63.0 + 1e-6)

    with tile.TileContext(nc) as tc:
        with (
            tc.tile_pool(name="wp", bufs=1) as wp,
            tc.tile_pool(name="hp", bufs=1) as hp,       # hT/A2all/Ball/h1T
            tc.tile_pool(name="h2p", bufs=4) as h2p,     # h2T per item
            tc.tile_pool(name="trp", bufs=3) as trp,     # tree scratch
            tc.tile_pool(name="smp", bufs=3) as smp,     # small node tiles
            tc.tile_pool(name="nbp", bufs=4) as nbp,     # An2|Bn per item
            tc.tile_pool(name="h4p", bufs=4) as h4p,     # MLP4 activations
            tc.tile_pool(name="osp", bufs=3) as osp,     # packed output
            tc.tile_pool(name="psml", bufs=2, space="PSUM") as psml,   # 1 bank
            tc.tile_pool(name="pbig", bufs=2, space="PSUM") as pbig,   # 2 banks
            tc.tile_pool(name="ppo", bufs=1, space="PSUM") as ppo,     # 2 banks
        ):
            wall = wp.tile([H, WTOT], BF16)
            # split the weight DMA so MLP1 can start before the tail lands
            nc.sync.dma_start(wall[:, 0:1024], wpack_d[:, 0:1024])
            nc.sync.dma_start(wall[:, 1024:WTOT], wpack_d[:, 1024:WTOT])
            bt = wp.tile([H, 8], F32)
            nc.sync.dma_start(bt, bias_d)
            xT = wall[0:N_IN, 0:XCOLS]
            w = {n: wall[:, XCOLS + H * i:XCOLS + H * (i + 1)]
                 for i, n in enumerate(WNAMES)}
            w["w1a"] = w["w1a"][0:N_IN, :]
            w["w4oz"] = w["w4oz"][:, 0:32]
            b = {n: bt[:, i:i + 1] for i, n in enumerate(BNAMES)}

            # ---- MLP1 over all 512 tokens ----
            p1 = psml.tile([H, XCOLS], F32, tag="psml")
            nc.tensor.matmul(p1, w["w1a"], xT, start=True, stop=True)
            h1T = hp.tile([H, XCOLS], BF16, tag="h1T")
            nc.scalar.activation(h1T, p1, _AF.Relu, bias=b["b1a"])
            p2 = psml.tile([H, XCOLS], F32, tag="psml")
            nc.tensor.matmul(p2, w["w1b"], h1T, start=True, stop=True)
            hT = hp.tile([H, XCOLS], BF16, tag="hT")
            nc.scalar.activation(hT, p2, _AF.Identity, bias=b["b1b"])

            # ---- A/B node projections for MLP2, all items at once ----
            pA = psml.tile([H, XCOLS], F32, tag="psml")
            nc.tensor.matmul(pA, w["w2s"], hT, start=True, stop=True)
            pB = psml.tile([H, XCOLS], F32, tag="psml")
            nc.tensor.matmul(pB, w["w2r"], hT, start=True, stop=True)
            # A2all: per item the 64 A-columns duplicated -> [A_b A_b]
            A2all = hp.tile([H, 2 * XCOLS], BF16, tag="A2all")
            nc.scalar.activation(
                _ap(A2all, 0, [[128, B_LOC], [64, 2], [1, 64]]),
                _ap(pA, 0, [[64, B_LOC], [0, 2], [1, 64]]),
                _AF.Identity, bias=b["b2a"],
            )
            Ball = hp.tile([H, XCOLS], BF16, tag="Ball")
            nc.scalar.activation(Ball, pB, _AF.Identity, bias=0.0)

            def tree(h2T):
                T = trp.tile([H, 2048], BF16, tag="T")
                nc.vector.tensor_add(
                    T[:, 0:1984], h2T[:, 0:1984], h2T[:, 2048:4032])
                nc.vector.tensor_copy(T[:, 1984:2048], h2T[:, 1984:2048])
                ww = 1024
                while ww >= N_NODES:
                    nc.vector.tensor_add(T[:, 0:ww], T[:, 0:ww], T[:, ww:2 * ww])
                    ww //= 2
                return T

            def stage1(bb):
                """edge expansion + relu + edge2node tree for item bb (DVE)."""
                h2T = h2p.tile([H, N_EDGES], BF16, tag="h2T")
                nc.vector.tensor_tensor(
                    _ap(h2T, 0, [[64, 63], [1, 64]]),
                    _ap(A2all, 128 * bb + 1, [[1, 63], [1, 64]]),
                    _ap(Ball, 64 * bb, [[0, 63], [1, 64]]),
                    _ALU.add,
                )
                nc.vector.tensor_scalar_max(h2T[:, :], h2T[:, :], 0.0)
                return h2T, tree(h2T)

            def stage1_pe(bb):
                """Pipeline-fill variant: expansion + relu on PE/ACT (idle
                during fill), only the tree on DVE."""
                h2T = h2p.tile([H, N_EDGES], BF16, tag="h2T")
                for half in range(2):
                    pms = [pbig.tile([H, 1024], F32, tag="pbig", name="pmx")
                           for _ in range(2)]
                    for i in range(4):
                        c = 4 * half + i
                        sl = pms[i // 2][:, 512 * (i % 2):512 * (i % 2 + 1)]
                        d0 = CHUNKS[c]
                        nc.tensor.matmul(
                            sl, w["ident"],
                            _ap(A2all, 128 * bb + d0, [[1, 8], [1, 64]]),
                            start=True, stop=False, skip_group_check=True,
                        )
                        nc.tensor.matmul(
                            sl, w["ident"],
                            _ap(Ball, 64 * bb, [[0, 8], [1, 64]]),
                            start=False, stop=True, skip_group_check=True,
                        )
                    if half == 0:
                        nc.scalar.activation(h2T[:, 0:1024], pms[0],
                                             _AF.Relu, bias=0.0)
                        nc.scalar.activation(h2T[:, 1024:2048], pms[1],
                                             _AF.Relu, bias=0.0)
                    else:
                        nc.scalar.activation(h2T[:, 2048:3072], pms[0],
                                             _AF.Relu, bias=0.0)
                        nc.scalar.activation(h2T[:, 3072:3584], pms[1][:, 0:512],
                                             _AF.Relu, bias=0.0)
                        nc.scalar.activation(h2T[:, 3520:4032], pms[1][:, 512:1024],
                                             _AF.Relu, bias=0.0)
                return h2T, tree(h2T)

            def stage2(bb, T, out):
                """edge2node tail + MLP3 + An/Bn projections for item bb.
                Generator: yields between serial steps so the emitter can
                interleave stage3 big matmuls into the PE/ACT queues."""
                pagg = psml.tile([H, N_NODES], F32, tag="psml")
                nc.tensor.matmul(pagg, w["w2b"], T[:, 0:N_NODES],
                                 start=True, stop=True)
                aggT = smp.tile([H, N_NODES], BF16, tag="aggT")
                nc.scalar.activation(aggT, pagg, _AF.Identity,
                                     bias=b["b2n"], scale=scale2n)
                yield
                pn1 = psml.tile([H, N_NODES], F32, tag="psml")
                nc.tensor.matmul(pn1, w["w3a"], aggT, start=True, stop=True)
                n1T = smp.tile([H, N_NODES], BF16, tag="n1T")
                nc.scalar.activation(n1T, pn1, _AF.Relu, bias=b["b3a"])
                yield
                pn2 = psml.tile([H, N_NODES], F32, tag="psml")
                nc.tensor.matmul(pn2, w["w3b"], n1T, start=True, stop=True)
                nT = smp.tile([H, N_NODES], BF16, tag="nT")
                nc.scalar.activation(nT, pn2, _AF.Identity, bias=b["b3b"])
                yield
                pAn = psml.tile([H, 128], F32, tag="psml")
                nc.tensor.matmul(pAn[:, 0:64], w["w4s"], nT, start=True, stop=True)
                nc.tensor.matmul(pAn[:, 64:128], w["w4r"], nT, start=True, stop=True)
                # AnBn: cols 0:128 = [An An] (with bias b4a'), cols 128:192 = Bn
                AnBn = nbp.tile([H, 192], BF16, tag="AnBn")
                nc.scalar.activation(
                    _ap(AnBn, 0, [[64, 2], [1, 64]]),
                    _ap(pAn, 0, [[0, 2], [1, 64]]),
                    _AF.Identity, bias=b["b4a"],
                )
                nc.scalar.activation(AnBn[:, 128:192], pAn[:, 64:128],
                                     _AF.Identity, bias=0.0)
                out[bb] = AnBn

            def stage3(bb, h2T, AnBn):
                """MLP4 + folded output layer for item bb.
                Generator: yields between matmul groups for interleaving."""
                outS = osp.tile([H, 1024], BF16, tag="outS")
                po = ppo.tile([H, 1024], F32, tag="ppo")
                for half in range(2):
                    pms = [pbig.tile([H, 1024], F32, tag="pbig", name="pm4")
                           for _ in range(2)]
                    cs = [4 * half + i for i in range(4)]
                    for i, c in enumerate(cs):
                        e0 = (CHUNKS[c] - 1) * 64
                        nc.tensor.matmul(
                            pms[i // 2][:, 512 * (i % 2):512 * (i % 2 + 1)],
                            w["w2bk"], h2T[:, e0:e0 + 512],
                            start=True, stop=False, skip_group_check=True,
                        )
                    yield
                    for i, c in enumerate(cs):
                        sl = pms[i // 2][:, 512 * (i % 2):512 * (i % 2 + 1)]
                        d0 = CHUNKS[c]
                        nc.tensor.matmul(
                            sl, w["ident"],
                            _ap(AnBn, d0, [[1, 8], [1, 64]]),
                            start=False, stop=False, skip_group_check=True,
                        )
                        nc.tensor.matmul(
                            sl, w["ident"],
                            _ap(AnBn, 128, [[0, 8], [1, 64]]),
                            start=False, stop=True, skip_group_check=True,
                        )
                    yield
                    h4s = []
                    for i in range(2):
                        h4 = h4p.tile([H, 1024], BF16, tag="h4")
                        nc.scalar.activation(h4, pms[i], _AF.Relu, bias=0.0)
                        h4s.append(h4)
                    yield
                    for k in range(4):
                        nc.tensor.matmul(
                            po[32 * k:32 * k + 32,
                               512 * half:512 * (half + 1)], w["w4oz"],
                            h4s[k // 2][:, 512 * (k % 2):512 * (k % 2 + 1)],
                            start=True, stop=True, tile_position=(0, 32 * k),
                        )
                    yield
                nc.scalar.activation(outS, po, _AF.Identity, bias=0.0)
                nc.sync.dma_start(y_d[bb], outS)

            # ---- software pipeline: stage2(b-1) | stage1(b) | D(b-1) |
            #      stage3(b-2); engines stay in-order without stalls.
            s1 = {}
            s2 = {}
            for cyc in range(B_LOC + 2):
                if cyc < B_LOC:
                    s1[cyc] = stage1_pe(cyc) if cyc < 2 else stage1(cyc)
                # interleave smalls(cyc-1) steps between big-matmul groups of
                # stage3(cyc-2) so PE keeps streaming during the small-matmul
                # ping-pong and ACT alternates small evacs with big evacs
                g2 = g3 = None
                if 1 <= cyc <= B_LOC:
                    g2 = stage2(cyc - 1, s1[cyc - 1][1], s2)
                if cyc >= 2:
                    bb = cyc - 2
                    g3 = stage3(bb, s1.pop(bb)[0], s2.pop(bb))
                while True:
                    done = True
                    if g2 is not None:
                        try:
                            next(g2)
                            done = False
                        except StopIteration:
                            g2 = None
                    if g3 is not None:
                        try:
                            next(g3)
                            done = False
                        except StopIteration:
                            g3 = None
                    if done:
                        break

    nc.compile()
    return nc


_CACHE = {}


def _get_nc():
    if "nc" not in _CACHE:
        _CACHE["nc"] = build_kernel()
        _CACHE["perm"] = _edge_perm()
    return _CACHE["nc"], _CACHE["perm"]


def make_in_maps(inputs):
    w2b = np.asarray(inputs["w2b"], np.float32)
    w4a = np.asarray(inputs["w4a"], np.float32)
    b2b = np.asarray(inputs["b2b"], np.float32)
    w4a_k = w4a[2 * H:]

    def pad128(a):
        out = np.zeros((H, a.shape[1]), np.float32)
        out[:a.shape[0]] = a
        return out

    w4o = np.asarray(inputs["w4b"], np.float32) @ inputs["wout"]
    wblocks = [
        pad128(np.asarray(inputs["w1a"], np.float32)),
        inputs["w1b"], inputs["w2a"][:H], inputs["w2a"][H:], w2b,
        inputs["w3a"], inputs["w3b"], w4a[:H], w4a[H:2 * H], w2b @ w4a_k,
        np.pad(w4o, ((0, 0), (0, H - N_OUT))),
        np.eye(H, dtype=np.float32),
    ]
    scale2n = 1.0 / (63.0 + 1e-6)
    bcols = [
        inputs["b1a"], inputs["b1b"], inputs["b2a"],
        63.0 * b2b * scale2n, inputs["b3a"], inputs["b3b"],
        np.asarray(inputs["b4a"], np.float32) + b2b @ w4a_k,
        np.zeros(H, np.float32),
    ]
    wfix = np.concatenate(
        [np.ascontiguousarray(v, np.float32) for v in wblocks], axis=1)
    bias = np.stack([np.asarray(v, np.float32) for v in bcols], axis=1)
    bias = np.ascontiguousarray(bias)
    x = np.asarray(inputs["x"], np.float32)
    in_maps = []
    for c in range(N_CORES):
        xs = x[c * B_LOC:(c + 1) * B_LOC]
        xTp = pad128(xs.reshape(B_LOC * N_NODES, N_IN).T)
        wpack = np.concatenate([xTp, wfix], axis=1).astype(BFNP)
        in_maps.append({"wpack": np.ascontiguousarray(wpack), "bias": bias})
    return in_maps


def gather_out(results, perm, inputs):
    b4o = (np.asarray(inputs["b4b"], np.float32) @ inputs["wout"]
           + inputs["bout"]).astype(np.float32)  # [16]
    inv = np.empty_like(perm)
    inv[perm] = np.arange(N_EDGES)
    out = np.empty((BATCH, N_EDGES, N_OUT), np.float32)
    full = np.empty((B_LOC, N_EDGES, N_OUT), np.float32)
    for cr in range(N_CORES):
        y = np.asarray(results[cr]["y"]).astype(np.float32)  # [B_LOC,128,1024]
        for c in range(8):
            h, k = divmod(c, 4)
            e0 = (CHUNKS[c] - 1) * 64
            seg = y[:, 32 * k:32 * k + N_OUT, 512 * h:512 * (h + 1)]
            full[:, e0:e0 + 512, :] = seg.transpose(0, 2, 1)
        out[cr * B_LOC:(cr + 1) * B_LOC] = full[:, inv, :] + b4o
    return out


def kernel(**inputs):
    nc, perm = _get_nc()
    in_maps = make_in_maps(inputs)
    res = run_bass_kernel_spmd(nc, in_maps, core_ids=list(range(N_CORES)))
    return gather_out(res.results, perm, inputs)
